# revision 1
# baseline (speedup 1.0000x reference)
"""Trainium2 Bass kernel for nn_DeepGcnV2 (GCNII-style message passing).

Data-parallel over the batch of 32 graphs: 4 graphs per NeuronCore on 8
cores.

Math: per layer  z = relu(LN(h));  s = (1-a)*Ahat@z + a*z;  h += s @ Wt
with Ahat = D^-1/2 (A+I) D^-1/2 and Wt = (1-beta) I + beta W.

Key restructuring vs a direct port: the whole normalized propagation
matrix is folded on the HOST into a single bf16 operand

    B = 0.9 * Ahat + 0.1 * I          (shipped as B^T, j-major rows)

so the device per layer does just two dense matmul groups and zero
transposes:

    s^T = z^T @ B^T     (lhsT = z node-major tiles, rhs = B^T streaming,
                         psum out is feature-major [H, N])
    h  += s @ Wt        (lhsT = s^T slices, rhs = Wt -> node-major psum)

LN statistics are free-dim reductions in node-major layout.  Degree
computation, dinv scalings, and the GCNII residual coefficients all
live inside B, so the device does no per-node elementwise scaling.
"""

import sys

for _p in ("/opt/trn_rl_repo",):
    if _p not in sys.path:
        sys.path.insert(0, _p)

import math

import ml_dtypes
import numpy as np

import concourse.bacc as bacc
import concourse.mybir as mybir
import concourse.tile as tile
from concourse.bass_types import AP
from concourse.bass_utils import run_bass_kernel_spmd

# Problem constants (hardcoded per contract)
BS, N, D, H, L = 32, 2048, 128, 64, 4
ALPHA, LAMDA, EPS = 0.1, 1.0, 1e-5
NCORES = 8
G = BS // NCORES          # graphs per core
P = 128                   # partitions
NT = N // P               # node tiles per graph
HALF = NT // 2            # tiles per psum half-batch
NCH = 4                   # aggregation psum chunks
CHW = N // NCH            # chunk width (512)

f32 = mybir.dt.float32
bf16 = mybir.dt.bfloat16


def _bmid(ap, n):
    """[128, F] AP -> [128, n, F] AP broadcast along an inserted middle dim."""
    dims = list(map(list, ap.ap))
    assert len(dims) == 2, dims
    return AP(ap.tensor, ap.offset, [dims[0], [0, n], dims[1]])


def _build_nc(trivial_affine=True, reps=1, graphs=G, skip_agg=False, skip_w=False,
              col_tile=True, skip_ln=False):
    nc = bacc.Bacc("TRN2", target_bir_lowering=False, debug=False)

    bt = nc.dram_tensor("bt", [G, N, N], bf16, kind="ExternalInput").ap()
    xt = nc.dram_tensor("xt", [G, D, N], bf16, kind="ExternalInput").ap()
    # pk slot 0: proj_w [128, H]; slot 1+l: Wt[l] in rows 0:64
    pk = nc.dram_tensor("pk", [P, 1 + L, H], bf16, kind="ExternalInput").ap()
    hwv = nc.dram_tensor("hwv", [P, H], f32, kind="ExternalInput").ap()
    if not trivial_affine:
        # slots: 0..L-1 ln_g, L..2L-1 ln_b, 2L proj_b, 2L+1 head_b(col 0)
        aux = nc.dram_tensor("aux", [P, 2 * L + 2, H], f32, kind="ExternalInput").ap()
    out = nc.dram_tensor("out", [G, N, 1], f32, kind="ExternalOutput").ap()

    AX = mybir.AxisListType.X
    ADD = mybir.AluOpType.add
    MUL = mybir.AluOpType.mult
    SUB = mybir.AluOpType.subtract
    AF = mybir.ActivationFunctionType

    from contextlib import ExitStack
    with tile.TileContext(nc) as tc, ExitStack() as ctx:
        ep = ctx.enter_context
        cpool = ep(tc.tile_pool(name="const", bufs=1))
        prm = ep(tc.tile_pool(name="prm", bufs=1))
        bt_pool = ep(tc.tile_pool(name="bt", bufs=2))
        xt_pool = ep(tc.tile_pool(name="xts", bufs=2))
        h_pool = ep(tc.tile_pool(name="h", bufs=2))
        z_pool = ep(tc.tile_pool(name="z", bufs=2))
        st_pool = ep(tc.tile_pool(name="st", bufs=2))
        big_pool = ep(tc.tile_pool(name="big", bufs=2))
        sm_pool = ep(tc.tile_pool(name="sm", bufs=4))
        ob_pool = ep(tc.tile_pool(name="ob", bufs=2))
        ps_s = ep(tc.tile_pool(name="ps_s", bufs=2, space="PSUM"))
        ps_w = ep(tc.tile_pool(name="ps_w", bufs=2, space="PSUM"))

        eps_sb = cpool.tile([P, 1], f32)
        nc.vector.memset(eps_sb[:], EPS)

        pk_sb = prm.tile([P, 1 + L, H], bf16)
        nc.sync.dma_start(pk_sb[:], pk)
        hwv_sb = prm.tile([P, H], f32)
        nc.sync.dma_start(hwv_sb[:], hwv)
        if not trivial_affine:
            aux_sb = prm.tile([P, 2 * L + 2, H], f32)
            nc.sync.dma_start(aux_sb[:], aux)

        for g in [gg for _ in range(reps) for gg in range(graphs)]:
            # ---- load B^T (bf16) and x^T (bf16) ----
            bt_sb = bt_pool.tile([P, NT, N], bf16)
            nc.sync.dma_start(
                bt_sb[:], bt[g].rearrange("(jb p) i -> p jb i", p=P))
            xt_sb = xt_pool.tile([D, N], bf16)
            nc.sync.dma_start(xt_sb[:], xt[g])

            # ---- h0 = x @ proj_w (+ proj_b) ----
            h = h_pool.tile([P, NT, H], f32)
            for half in range(2):
                hp = ps_w.tile([P, HALF, H], f32)
                for q in range(HALF):
                    i = half * HALF + q
                    nc.tensor.matmul(
                        hp[:, q, :], lhsT=xt_sb[:, i * P:(i + 1) * P],
                        rhs=pk_sb[:, 0, :], start=True, stop=True)
                hs = slice(half * HALF, (half + 1) * HALF)
                if trivial_affine:
                    nc.vector.tensor_copy(h[:, hs, :], hp[:, :, :])
                else:
                    nc.vector.tensor_tensor(
                        h[:, hs, :], hp[:, :, :],
                        _bmid(aux_sb[:, 2 * L, :], HALF), op=ADD)

            # ---- layers ----
            for l in range(L):
                z = z_pool.tile([P, NT, H], bf16)
                if skip_ln:
                    nc.scalar.activation(z[:, :, :], h[:, :, :], AF.Relu)
                else:
                  # LN stats: mu, rstd per node
                  if True:
                    musum = sm_pool.tile([P, NT], f32, tag="musum")
                  nc.vector.tensor_reduce(musum[:], h[:, :, :], axis=AX, op=ADD)
                  sqh = big_pool.tile([P, NT, H], f32, tag="big")
                  nc.scalar.activation(sqh[:], h[:, :, :], AF.Square)
                  ssq = sm_pool.tile([P, NT], f32, tag="ssq")
                  nc.vector.tensor_reduce(ssq[:], sqh[:, :, :], axis=AX, op=ADD)
                  mu = sm_pool.tile([P, NT], f32, tag="mu")
                  nc.vector.tensor_scalar(mu[:], musum[:], 1.0 / H, None, MUL)
                  var = sm_pool.tile([P, NT], f32, tag="var")
                  nc.vector.tensor_tensor(var[:], mu[:], mu[:], op=MUL)
                  nc.vector.tensor_scalar(ssq[:], ssq[:], 1.0 / H, None, MUL)
                  nc.vector.tensor_tensor(var[:], ssq[:], var[:], op=SUB)
                  stdv = sm_pool.tile([P, NT], f32, tag="stdv")
                  nc.scalar.activation(stdv[:], var[:], AF.Sqrt, bias=eps_sb[:, 0:1])
                  rstd = sm_pool.tile([P, NT], f32, tag="rstd")
                  nc.vector.reciprocal(rstd[:], stdv[:])

                  # z = relu((h - mu) * rstd [* g + b])  (bf16, node-major)
                  # elementwise work on the otherwise-idle GpSimd engine
                  zf = big_pool.tile([P, NT, H], f32, tag="big")
                  nc.gpsimd.tensor_tensor(
                      zf[:, :, :], h[:, :, :], mu[:].broadcast_to([P, NT, H]), op=SUB)
                  nc.gpsimd.tensor_tensor(
                      zf[:, :, :], zf[:, :, :], rstd[:].broadcast_to([P, NT, H]), op=MUL)
                  if not trivial_affine:
                      nc.vector.tensor_tensor(
                          zf[:, :, :], zf[:, :, :], _bmid(aux_sb[:, l, :], NT), op=MUL)
                      nc.vector.tensor_tensor(
                          zf[:, :, :], zf[:, :, :], _bmid(aux_sb[:, L + l, :], NT), op=ADD)
                  nc.scalar.activation(z[:, :, :], zf[:, :, :], AF.Relu)

                # s^T = z^T @ B^T, col-tiled 2x: PE cols 0:63 (chain A) compute
                # node-columns 0:1023 -> psum rows 0:64; cols 64:127 (chain B)
                # compute 1024:2047 -> psum rows 64:128.  Both chains share the
                # z weights and run concurrently.
                # st layout: [0:64, 0:1024] = s^T[:, 0:1024];
                #            [64:128, 0:1024] = s^T[:, 1024:2048]
                if col_tile:
                    st = st_pool.tile([P, N // 2], bf16, tag="st")
                else:
                    st = st_pool.tile([H, N], bf16, tag="st")
                if skip_agg:
                    nc.vector.memset(st[:, :], 0.0)
                elif col_tile:
                    pss = ps_s.tile([P, N // 2], f32)
                    for cp in range(2):   # chunk pair: A chunk cp, B chunk 2+cp
                        csA = slice(cp * CHW, (cp + 1) * CHW)
                        csB = slice((2 + cp) * CHW, (3 + cp) * CHW)
                        csP = slice(cp * CHW, (cp + 1) * CHW)
                        for jb in range(NT):
                            nc.tensor.matmul(
                                pss[0:H, csP], lhsT=z[:, jb, :],
                                rhs=bt_sb[:, jb, csA],
                                start=(jb == 0), stop=(jb == NT - 1))
                            nc.tensor.matmul(
                                pss[H:P, csP], lhsT=z[:, jb, :],
                                rhs=bt_sb[:, jb, csB],
                                start=(jb == 0), stop=(jb == NT - 1))
                        # copy each half-chunk out as it completes; full-width
                        # [128, 512] copies cover both chains in one op
                        if cp == 0:
                            nc.vector.tensor_copy(st[:, csP], pss[:, csP])
                        else:
                            nc.scalar.copy(st[:, csP], pss[:, csP])
                else:
                    # single-chain layout: psum [H, 1024] tiles, st [H, N]
                    for half in range(2):
                        pss = ps_s.tile([P, N // 2], f32)
                        for cp in range(2):
                            c = half * 2 + cp
                            cs = slice(c * CHW, (c + 1) * CHW)
                            csP = slice(cp * CHW, (cp + 1) * CHW)
                            for jb in range(NT):
                                nc.tensor.matmul(
                                    pss[0:H, csP], lhsT=z[:, jb, :],
                                    rhs=bt_sb[:, jb, cs],
                                    start=(jb == 0), stop=(jb == NT - 1))
                            if cp == 0:
                                nc.vector.tensor_copy(st[:, cs], pss[0:H, csP])
                            else:
                                nc.scalar.copy(st[:, cs], pss[0:H, csP])

                # h += s @ Wt[l]   (lhsT = s^T slices -> node-major psum);
                # node tiles i<8 read st rows 0:64 (PE rows 0:63), i>=8 read
                # rows 64:128 (PE rows 64:127) -> row-tiled concurrency.
                for half in range(0 if skip_w else 2):
                    wp = ps_w.tile([P, HALF, H], f32)
                    for q in range(HALF):
                        i = half * HALF + q
                        if not col_tile:
                            lhsT = st[:, i * P:(i + 1) * P]
                            rhs = pk_sb[0:H, 1 + l, :]
                        elif i < NT // 2:
                            lhsT = st[0:H, i * P:(i + 1) * P]
                            rhs = pk_sb[0:H, 1 + l, :]
                        else:
                            j = i - NT // 2
                            lhsT = st[H:P, j * P:(j + 1) * P]
                            rhs = pk_sb[H:P, 1 + l, :]
                        nc.tensor.matmul(
                            wp[:, q, :], lhsT=lhsT, rhs=rhs,
                            start=True, stop=True)
                    hs = slice(half * HALF, (half + 1) * HALF)
                    nc.vector.tensor_tensor(
                        h[:, hs, :], h[:, hs, :], wp[:, :, :], op=ADD)

            # ---- head: out = h @ head_w (+ head_b) ----
            th = big_pool.tile([P, NT, H], f32, tag="big")
            nc.gpsimd.tensor_tensor(
                th[:, :, :], h[:, :, :], _bmid(hwv_sb[:, :], NT), op=MUL)
            osb = ob_pool.tile([P, NT], f32)
            nc.vector.tensor_reduce(osb[:], th[:, :, :], axis=AX, op=ADD)
            if not trivial_affine:
                nc.vector.tensor_scalar(
                    osb[:], osb[:], aux_sb[:, 2 * L + 1, 0:1], None, ADD)
            nc.sync.dma_start(
                out=out[g].rearrange("(ib p) one -> p (ib one)", p=P),
                in_=osb[:, :])

    nc.compile()
    return nc


_NC = {}


def _get_nc(trivial_affine=True):
    key = trivial_affine
    if key not in _NC:
        _NC[key] = _build_nc(trivial_affine=trivial_affine)
    return _NC[key]


def _prep_in_maps(inputs):
    x = np.asarray(inputs["x"], np.float32)
    adj = np.asarray(inputs["adj"], np.float32)
    proj_w = np.asarray(inputs["proj_w"], np.float32)
    proj_b = np.asarray(inputs["proj_b"], np.float32)
    ln_g = np.asarray(inputs["ln_g"], np.float32)
    ln_b = np.asarray(inputs["ln_b"], np.float32)
    conv_w = np.asarray(inputs["conv_w"], np.float32)
    head_w = np.asarray(inputs["head_w"], np.float32)
    head_b = np.asarray(inputs["head_b"], np.float32)

    trivial_affine = bool(
        np.all(ln_g == 1.0) and np.all(ln_b == 0.0)
        and np.all(proj_b == 0.0) and np.all(head_b == 0.0))

    # Wt[l] = (1-beta) I + beta conv_w[l], replicated in both partition
    # halves (row-tiled w-matmuls read rows 0:64 or 64:128)
    pkh = np.zeros((P, 1 + L, H), np.float32)
    pkh[:, 0, :] = proj_w
    for l in range(L):
        beta = math.log(LAMDA / (l + 1) + 1.0)
        wt = (1.0 - beta) * np.eye(H, dtype=np.float32) + beta * conv_w[l]
        pkh[:H, 1 + l, :] = wt
        pkh[H:, 1 + l, :] = wt

    # B^T per graph, bf16: bt[j, i] = 0.9*d_j*d_i*((A!=0)^T + I)[j,i] + 0.1*I
    ey = np.eye(N, dtype=np.float32)
    bt_all = np.empty((BS, N, N), ml_dtypes.bfloat16)

    def _build_bt(gi):
        pat = adj[gi] != 0
        deg = pat.sum(-1, dtype=np.float32) + 1.0
        sc = np.sqrt(0.9 / deg)                         # sqrt(0.9) * deg^-1/2
        b = pat.T.astype(np.float32)
        b += ey
        b *= sc[:, None]
        b *= sc[None, :]
        b[np.arange(N), np.arange(N)] += 0.1
        bt_all[gi] = b.astype(ml_dtypes.bfloat16)

    from concurrent.futures import ThreadPoolExecutor
    with ThreadPoolExecutor(max_workers=8) as ex:
        list(ex.map(_build_bt, range(BS)))

    xt_all = np.ascontiguousarray(
        x.transpose(0, 2, 1)).astype(ml_dtypes.bfloat16)

    shared = {
        "pk": pkh.astype(ml_dtypes.bfloat16),
        "hwv": np.ascontiguousarray(
            np.broadcast_to(head_w[:, 0][None, :], (P, H))),
    }
    if not trivial_affine:
        aux = np.zeros((P, 2 * L + 2, H), np.float32)
        aux[:, :L, :] = np.broadcast_to(ln_g[:, None, :], (L, P, H)).transpose(1, 0, 2)
        aux[:, L:2 * L, :] = np.broadcast_to(ln_b[:, None, :], (L, P, H)).transpose(1, 0, 2)
        aux[:, 2 * L, :] = proj_b[None, :]
        aux[:, 2 * L + 1, 0] = head_b[0]
        shared["aux"] = aux

    in_maps = []
    for c in range(NCORES):
        sl = slice(c * G, (c + 1) * G)
        in_maps.append(dict(
            shared,
            bt=np.ascontiguousarray(bt_all[sl]),
            xt=np.ascontiguousarray(xt_all[sl]),
        ))
    return in_maps, trivial_affine


def kernel(**inputs) -> np.ndarray:
    in_maps, trivial_affine = _prep_in_maps(inputs)
    nc = _get_nc(trivial_affine)
    res = run_bass_kernel_spmd(nc, in_maps, list(range(NCORES)))
    return np.concatenate([res.results[c]["out"] for c in range(NCORES)], axis=0)



# revision 26
# speedup vs baseline: 1.4800x; 1.4800x over previous
"""Trainium2 Bass kernel for nn_DeepGcnV2 (GCNII-style message passing).

Data-parallel over the batch of 32 graphs: 4 graphs per NeuronCore on 8
cores.

Math: per layer  z = relu(LN(h));  s = (1-a)*Ahat@z + a*z;  h += s @ Wt
with Ahat = D^-1/2 (A+I) D^-1/2 and Wt = (1-beta) I + beta W.

The whole normalized propagation matrix is folded on the HOST into a
single bf16 operand

    B = 0.9 * Ahat + 0.1 * I          (shipped as B^T, j-major rows)

so the device per layer does two dense matmul groups and zero
transposes:

    s^T = z^T @ B^T     (lhsT = z node-major tiles, rhs = B^T streaming,
                         psum out is feature-major)
    h  += s @ Wt        (lhsT = s^T slices, rhs = Wt, accumulated
                         DIRECTLY into the h PSUM region)

Pipelining structure (the fast path):
  - h lives in PSUM for the graph's whole lifetime; h0 = x @ proj_w
    starts the accumulation groups and every layer's s @ Wt matmuls
    accumulate in place (start=False).  No DVE h-updates at all.
  - B^T columns are permuted host-side so the first 512-column
    aggregation chunk (both PE column-tiled chains) yields s^T for
    nodes 0:1023 = h bank 0, and the second chunk nodes 1024:2047 =
    h bank 1.  After each chunk: psum->sbuf copy, eight s@Wt matmuls
    into one h bank, then LN stats (bn_stats) + fused
    relu(h*rstd - mu*rstd) scalar-engine ops produce the NEXT layer's
    z tiles for that bank -- all while the other chunk's matmuls keep
    the PE busy.  This keeps the tensor engine dense (HAM warm) and
    hides the LN serial chain entirely.
"""

import sys

for _p in ("/opt/trn_rl_repo",):
    if _p not in sys.path:
        sys.path.insert(0, _p)

import math

import ml_dtypes
import numpy as np

import concourse.bacc as bacc
import concourse.mybir as mybir
import concourse.tile as tile
from concourse.bass_types import AP
from concourse.bass_utils import run_bass_kernel_spmd

# Problem constants (hardcoded per contract)
BS, N, D, H, L = 32, 2048, 128, 64, 4
ALPHA, LAMDA, EPS = 0.1, 1.0, 1e-5
NCORES = 8
G = BS // NCORES          # graphs per core
P = 128                   # partitions
NT = N // P               # node tiles per graph
HALF = NT // 2            # tiles per psum half-batch
NCH = 4                   # aggregation psum chunks
CHW = N // NCH            # chunk width (512)
NH = N // 2               # 1024
# CoreSim group-check workaround: its zero-region tracker ignores the
# PSUM base partition, so the dual-chain col-tiled aggregation trips a
# false "pending group" error.  Hardware semantics are unaffected.
SKIP_GROUP_CHECK_FOR_SIM = False

f32 = mybir.dt.float32
bf16 = mybir.dt.bfloat16


def _bmid(ap, n):
    """[128, F] AP -> [128, n, F] AP broadcast along an inserted middle dim."""
    dims = list(map(list, ap.ap))
    assert len(dims) == 2, dims
    return AP(ap.tensor, ap.offset, [dims[0], [0, n], dims[1]])


def _build_nc_fast(reps=1, graphs=G, h_in_psum=True, dma_quarters=True,
                   fused_z=True, alt_wmm=True, head_dve=True, pipelined=True,
                   nlayers=L, do_head=True, do_h0_stats=True,
                   do_agg=True, do_wmm=True, sbuf_stats=True, wmm_mode='full'):
    """Fast path: trivial affine (ln_g=1, ln_b=0, proj_b=0, head_b=0)."""
    nc = bacc.Bacc("TRN2", target_bir_lowering=False, debug=False)

    # bt columns are HOST-PERMUTED: col c holds node perm[c] with
    # perm = [0:512, 1024:1536, 512:1024, 1536:2048]; see _prep_in_maps.
    bt = nc.dram_tensor("bt", [G, N, N], bf16, kind="ExternalInput").ap()
    xt = nc.dram_tensor("xt", [G, D, N], bf16, kind="ExternalInput").ap()
    # pk slot 0: proj_w [128, H]; slot 1+2l: [Wt[l]; 0], slot 2+2l: [0; Wt[l]]
    # (zero-padded so every s@Wt matmul is full-K with tile_position (0,0) --
    # concurrent row-group-tiled matmuls draining into one PSUM bank fault
    # the exec unit)
    pk = nc.dram_tensor("pk", [P, 1 + 2 * L, H], bf16, kind="ExternalInput").ap()
    hwv = nc.dram_tensor("hwv", [P, H], f32, kind="ExternalInput").ap()
    out = nc.dram_tensor("out", [G, N, 1], f32, kind="ExternalOutput").ap()

    AX = mybir.AxisListType.X
    ADD = mybir.AluOpType.add
    MUL = mybir.AluOpType.mult
    SUB = mybir.AluOpType.subtract
    AF = mybir.ActivationFunctionType

    from contextlib import ExitStack
    with tile.TileContext(nc) as tc, ExitStack() as ctx:
        ep = ctx.enter_context
        cpool = ep(tc.tile_pool(name="const", bufs=1))
        prm = ep(tc.tile_pool(name="prm", bufs=1))
        bt_pool = ep(tc.tile_pool(name="bt", bufs=2))
        xt_pool = ep(tc.tile_pool(name="xts", bufs=2))
        z_pool = ep(tc.tile_pool(name="z", bufs=2))
        st_pool = ep(tc.tile_pool(name="st", bufs=2))
        th_pool = ep(tc.tile_pool(name="th", bufs=2))
        bno_pool = ep(tc.tile_pool(name="bno", bufs=2))
        sm_pool = ep(tc.tile_pool(name="sm", bufs=4))
        ob_pool = ep(tc.tile_pool(name="ob", bufs=2))
        if h_in_psum:
            ps_h = ep(tc.tile_pool(name="ps_h", bufs=2, space="PSUM"))
        else:
            h_pool = ep(tc.tile_pool(name="h", bufs=2))
            ps_w = ep(tc.tile_pool(name="ps_w", bufs=2, space="PSUM"))
        ps_s = ep(tc.tile_pool(name="ps_s", bufs=2, space="PSUM"))

        eps_sb = cpool.tile([P, 1], f32)
        nc.vector.memset(eps_sb[:], EPS)

        pk_sb = prm.tile([P, 1 + 2 * L, H], bf16)
        nc.sync.dma_start(pk_sb[:], pk)
        hwv_sb = prm.tile([P, H], f32)
        nc.sync.dma_start(hwv_sb[:], hwv)

        def stats_and_z(h, z_out, half):
            """LN stats for h-bank `half` (tiles half*8..half*8+7) and the
            fused relu((h-mu)*rstd) producing z tiles for that bank.

            The first two reads of the bank (reduce + square) cover the
            bank's FULL byte range, so they depend on every matmul that
            wrote it -- no PSUM PE-W/engine-R same-bank overlap is
            possible regardless of scheduler order.  The per-tile z
            activations depend on rstd and are therefore also ordered
            after all of the bank's writes."""
            h8 = slice(half * HALF, (half + 1) * HALF)
            if sbuf_stats:
                # Evacuate the h bank to SBUF with a single full-range DVE
                # copy; every ScalarE op then reads SBUF only.  (TensorE
                # writing PSUM concurrently with ScalarE reading PSUM --
                # even a different bank -- faults the exec unit.)
                hv = th_pool.tile([P, HALF, H], f32, tag="hv")
                nc.vector.tensor_copy(hv[:, :, :], h[:, h8, :])
                hsrc = hv[:, :, :]
                def htile(q):
                    return hv[:, q, :]
            else:
                hsrc = h[:, h8, :]
                def htile(q):
                    return h[:, half * HALF + q, :]
            musum = sm_pool.tile([P, HALF], f32, tag="musum")
            nc.vector.tensor_reduce(musum[:], hsrc, axis=AX, op=ADD)
            sq = bno_pool.tile([P, HALF, H], f32, tag="sq")
            nc.scalar.activation(sq[:, :, :], hsrc, AF.Square)
            ssq = sm_pool.tile([P, HALF], f32, tag="ssq")
            nc.vector.tensor_reduce(ssq[:], sq[:, :, :], axis=AX, op=ADD)
            mun = sm_pool.tile([P, HALF], f32, tag="mun")
            nc.vector.tensor_scalar(mun[:], musum[:], -1.0 / H, None, MUL)
            e2 = sm_pool.tile([P, HALF], f32, tag="e2")
            nc.vector.tensor_scalar(e2[:], ssq[:], 1.0 / H, None, MUL)
            mu2 = sm_pool.tile([P, HALF], f32, tag="mu2")
            nc.vector.tensor_tensor(mu2[:], mun[:], mun[:], op=MUL)
            var = sm_pool.tile([P, HALF], f32, tag="var")
            nc.vector.tensor_tensor(var[:], e2[:], mu2[:], op=SUB)
            stdv = sm_pool.tile([P, HALF], f32, tag="stdv")
            nc.scalar.activation(stdv[:], var[:], AF.Sqrt, bias=eps_sb[:, 0:1])
            rstd = sm_pool.tile([P, HALF], f32, tag="rstd")
            nc.vector.reciprocal(rstd[:], stdv[:])
            nmr = sm_pool.tile([P, HALF], f32, tag="nmr")
            nc.vector.tensor_tensor(nmr[:], mun[:], rstd[:], op=MUL)
            if fused_z:
                for q in range(HALF):
                    i = half * HALF + q
                    nc.scalar.activation(
                        z_out[:, i, :], htile(q), AF.Relu,
                        bias=nmr[:, q:q + 1], scale=rstd[:, q:q + 1])
            else:
                zf = bno_pool.tile([P, HALF, H], f32, tag="zf")
                nc.gpsimd.tensor_tensor(
                    zf[:, :, :], hsrc,
                    mun[:].broadcast_to([P, HALF, H]), op=ADD)
                nc.gpsimd.tensor_tensor(
                    zf[:, :, :], zf[:, :, :],
                    rstd[:].broadcast_to([P, HALF, H]), op=MUL)
                nc.scalar.activation(z_out[:, h8, :], zf[:, :, :], AF.Relu)

        def head_half(h, osb, cp):
            # head half: out = h @ head_w (rowwise dot)
            h8 = slice(cp * HALF, (cp + 1) * HALF)
            th = th_pool.tile([P, HALF, H], f32, tag="th")
            if head_dve or h_in_psum:
                nc.vector.tensor_tensor(
                    th[:, :, :], h[:, h8, :],
                    _bmid(hwv_sb[:, :], HALF), op=MUL)
            else:
                nc.gpsimd.tensor_tensor(
                    th[:, :, :], h[:, h8, :],
                    _bmid(hwv_sb[:, :], HALF), op=MUL)
            nc.vector.tensor_reduce(
                osb[:, h8], th[:, :, :], axis=AX, op=ADD)

        for g in [gg for _ in range(reps) for gg in range(graphs)]:
            # ---- load B^T (bf16, quarter-chunked) and x^T (bf16) ----
            bt_sb = bt_pool.tile([P, NT, N], bf16)
            bt_src = bt[g].rearrange("(jb p) i -> p jb i", p=P)
            if dma_quarters:
                for q4 in range(4):
                    nc.sync.dma_start(
                        bt_sb[:, 4 * q4:4 * (q4 + 1), :],
                        bt_src[:, 4 * q4:4 * (q4 + 1), :])
            else:
                nc.sync.dma_start(bt_sb[:], bt_src)
            xt_sb = xt_pool.tile([D, N], bf16)
            nc.sync.dma_start(xt_sb[:], xt[g])

            # ---- h0 = x @ proj_w, accumulation groups start here ----
            if h_in_psum:
                h = ps_h.tile([P, NT, H], f32)
            else:
                h = h_pool.tile([P, NT, H], f32, tag="h")
            z_cur = z_pool.tile([P, NT, H], bf16, tag="z")
            for half in range(2):
                if h_in_psum:
                    for q in range(HALF):
                        i = half * HALF + q
                        # start opens the bank's accumulation (pending-zero)
                        # region; stop closes the sim's group bookkeeping so
                        # DVE/ACT may read h.  Later s@Wt matmuls accumulate
                        # with skip_group_check (stop is sim-only; hardware
                        # accumulation is driven by the has_written bits).
                        nc.tensor.matmul(
                            h[:, i, :], lhsT=xt_sb[:, i * P:(i + 1) * P],
                            rhs=pk_sb[:, 0, :], start=(q == 0),
                            stop=(q == HALF - 1))
                else:
                    hp = ps_w.tile([P, HALF, H], f32, tag="wp")
                    for q in range(HALF):
                        i = half * HALF + q
                        nc.tensor.matmul(
                            hp[:, q, :], lhsT=xt_sb[:, i * P:(i + 1) * P],
                            rhs=pk_sb[:, 0, :], start=True, stop=True)
                    hs = slice(half * HALF, (half + 1) * HALF)
                    nc.vector.tensor_copy(h[:, hs, :], hp[:, :, :])
                if pipelined and do_h0_stats:
                    stats_and_z(h, z_cur, half)
            if not pipelined and do_h0_stats:
                stats_and_z(h, z_cur, 0)
                stats_and_z(h, z_cur, 1)
            if not do_h0_stats:
                nc.vector.memset(z_cur[:, :, :], 0.01)

            # ---- layers ----
            for l in range(nlayers):
                last_l = l == nlayers - 1
                z_next = None
                if not last_l:
                    z_next = z_pool.tile([P, NT, H], bf16, tag="z")
                st = st_pool.tile([P, NH], bf16, tag="st")
                pss = ps_s.tile([P, NH], f32)
                osb = None
                if last_l:
                    osb = ob_pool.tile([P, NT], f32, tag="osb")
                    if not do_head:
                        nc.vector.memset(osb[:], 0.0)
                for cp in range(2):
                    cs = slice(cp * CHW, (cp + 1) * CHW)
                    csB = slice(N // 2 + cp * CHW, N // 2 + (cp + 1) * CHW)
                    # s^T chunk: dual PE column-tiled chains; chunk cp
                    # covers nodes cp*1024 .. cp*1024+1023 (h bank cp)
                    if do_agg:
                        for jb in range(NT):
                            nc.tensor.matmul(
                                pss[0:H, cs], lhsT=z_cur[:, jb, :],
                                rhs=bt_sb[:, jb, cs],
                                start=(jb == 0), stop=(jb == NT - 1))
                            nc.tensor.matmul(
                                pss[H:P, cs], lhsT=z_cur[:, jb, :],
                                rhs=bt_sb[:, jb, csB],
                                start=(jb == 0), stop=(jb == NT - 1),
                                skip_group_check=SKIP_GROUP_CHECK_FOR_SIM)
                        if cp == 0 or sbuf_stats:
                            nc.vector.tensor_copy(st[:, cs], pss[:, cs])
                        else:
                            nc.scalar.copy(st[:, cs], pss[:, cs])
                    else:
                        nc.vector.memset(st[:, cs], 0.01)
                    # h-bank cp: h += s @ Wt for tiles cp*8 .. cp*8+7,
                    # alternating PE row groups for LDW/MM overlap
                    if not do_wmm:
                        pass
                    elif wmm_mode == 'fromxt':
                        for k in range(4):
                            i = cp * HALF + k
                            nc.tensor.matmul(
                                h[:, i, :], lhsT=xt_sb[:, i * P:(i + 1) * P],
                                rhs=pk_sb[:, 1 + 2 * l + 0, :],
                                start=False, stop=False,
                                skip_group_check=True)
                    elif h_in_psum:
                        # full-K matmuls: st rows 0:64 = chain A (nodes
                        # cp*1024+k*128..), rows 64:128 = chain B (nodes
                        # cp*1024+512+k*128..); the zero half of the
                        # padded Wt slot kills the other chain's rows.
                        for k in range(4):
                            for rh in range(2):
                                i = cp * HALF + rh * 4 + k
                                nc.tensor.matmul(
                                    h[:, i, :],
                                    lhsT=st[:, cp * CHW + k * P:
                                            cp * CHW + (k + 1) * P],
                                    rhs=pk_sb[:, 1 + 2 * l + rh, :],
                                    start=False, stop=False,
                                    skip_group_check=True)
                    else:
                        wp = ps_w.tile([P, HALF, H], f32, tag="wp")
                        for k in range(4):
                            for rh in range(2):
                                i = cp * HALF + rh * 4 + k
                                q = rh * 4 + k
                                nc.tensor.matmul(
                                    wp[:, q, :],
                                    lhsT=st[:, cp * CHW + k * P:
                                            cp * CHW + (k + 1) * P],
                                    rhs=pk_sb[:, 1 + 2 * l + rh, :],
                                    start=True, stop=True)
                        hs = slice(cp * HALF, (cp + 1) * HALF)
                        # wp tile order: [0..3]=tiles cp8+0..3, [4..7]=cp8+4..7
                        nc.vector.tensor_tensor(
                            h[:, hs, :], h[:, hs, :], wp[:, :, :], op=ADD)
                    if pipelined:
                        if not last_l:
                            stats_and_z(h, z_next, cp)
                        elif do_head:
                            head_half(h, osb, cp)
                if not pipelined:
                    for cp in range(2):
                        if not last_l:
                            stats_and_z(h, z_next, cp)
                        elif do_head:
                            head_half(h, osb, cp)
                z_cur = z_next

            nc.sync.dma_start(
                out=out[g].rearrange("(ib p) one -> p (ib one)", p=P),
                in_=osb[:, :])

    nc.compile()
    return nc


def _build_nc_general(trivial_affine=False, reps=1, graphs=G):
    """General path (non-trivial affine); also works for trivial."""
    nc = bacc.Bacc("TRN2", target_bir_lowering=False, debug=False)

    bt = nc.dram_tensor("bt", [G, N, N], bf16, kind="ExternalInput").ap()
    xt = nc.dram_tensor("xt", [G, D, N], bf16, kind="ExternalInput").ap()
    pk = nc.dram_tensor("pk", [P, 1 + L, H], bf16, kind="ExternalInput").ap()
    hwv = nc.dram_tensor("hwv", [P, H], f32, kind="ExternalInput").ap()
    if not trivial_affine:
        # slots: 0..L-1 ln_g, L..2L-1 ln_b, 2L proj_b, 2L+1 head_b(col 0)
        aux = nc.dram_tensor("aux", [P, 2 * L + 2, H], f32, kind="ExternalInput").ap()
    out = nc.dram_tensor("out", [G, N, 1], f32, kind="ExternalOutput").ap()

    AX = mybir.AxisListType.X
    ADD = mybir.AluOpType.add
    MUL = mybir.AluOpType.mult
    SUB = mybir.AluOpType.subtract
    AF = mybir.ActivationFunctionType

    from contextlib import ExitStack
    with tile.TileContext(nc) as tc, ExitStack() as ctx:
        ep = ctx.enter_context
        cpool = ep(tc.tile_pool(name="const", bufs=1))
        prm = ep(tc.tile_pool(name="prm", bufs=1))
        bt_pool = ep(tc.tile_pool(name="bt", bufs=2))
        xt_pool = ep(tc.tile_pool(name="xts", bufs=2))
        h_pool = ep(tc.tile_pool(name="h", bufs=2))
        z_pool = ep(tc.tile_pool(name="z", bufs=2))
        st_pool = ep(tc.tile_pool(name="st", bufs=2))
        big_pool = ep(tc.tile_pool(name="big", bufs=2))
        sm_pool = ep(tc.tile_pool(name="sm", bufs=4))
        ob_pool = ep(tc.tile_pool(name="ob", bufs=2))
        ps_s = ep(tc.tile_pool(name="ps_s", bufs=2, space="PSUM"))
        ps_w = ep(tc.tile_pool(name="ps_w", bufs=2, space="PSUM"))

        eps_sb = cpool.tile([P, 1], f32)
        nc.vector.memset(eps_sb[:], EPS)

        pk_sb = prm.tile([P, 1 + L, H], bf16)
        nc.sync.dma_start(pk_sb[:], pk)
        hwv_sb = prm.tile([P, H], f32)
        nc.sync.dma_start(hwv_sb[:], hwv)
        if not trivial_affine:
            aux_sb = prm.tile([P, 2 * L + 2, H], f32)
            nc.sync.dma_start(aux_sb[:], aux)

        for g in [gg for _ in range(reps) for gg in range(graphs)]:
            # ---- load B^T (bf16) and x^T (bf16) ----
            bt_sb = bt_pool.tile([P, NT, N], bf16)
            nc.sync.dma_start(
                bt_sb[:], bt[g].rearrange("(jb p) i -> p jb i", p=P))
            xt_sb = xt_pool.tile([D, N], bf16)
            nc.sync.dma_start(xt_sb[:], xt[g])

            # ---- h0 = x @ proj_w (+ proj_b) ----
            h = h_pool.tile([P, NT, H], f32)
            for half in range(2):
                hp = ps_w.tile([P, HALF, H], f32)
                for q in range(HALF):
                    i = half * HALF + q
                    nc.tensor.matmul(
                        hp[:, q, :], lhsT=xt_sb[:, i * P:(i + 1) * P],
                        rhs=pk_sb[:, 0, :], start=True, stop=True)
                hs = slice(half * HALF, (half + 1) * HALF)
                if trivial_affine:
                    nc.vector.tensor_copy(h[:, hs, :], hp[:, :, :])
                else:
                    nc.vector.tensor_tensor(
                        h[:, hs, :], hp[:, :, :],
                        _bmid(aux_sb[:, 2 * L, :], HALF), op=ADD)

            # ---- layers ----
            for l in range(L):
                z = z_pool.tile([P, NT, H], bf16)
                # LN stats: mu, rstd per node
                musum = sm_pool.tile([P, NT], f32, tag="musum")
                nc.vector.tensor_reduce(musum[:], h[:, :, :], axis=AX, op=ADD)
                sqh = big_pool.tile([P, NT, H], f32, tag="big")
                nc.scalar.activation(sqh[:], h[:, :, :], AF.Square)
                ssq = sm_pool.tile([P, NT], f32, tag="ssq")
                nc.vector.tensor_reduce(ssq[:], sqh[:, :, :], axis=AX, op=ADD)
                mu = sm_pool.tile([P, NT], f32, tag="mu")
                nc.vector.tensor_scalar(mu[:], musum[:], 1.0 / H, None, MUL)
                var = sm_pool.tile([P, NT], f32, tag="var")
                nc.vector.tensor_tensor(var[:], mu[:], mu[:], op=MUL)
                nc.vector.tensor_scalar(ssq[:], ssq[:], 1.0 / H, None, MUL)
                nc.vector.tensor_tensor(var[:], ssq[:], var[:], op=SUB)
                stdv = sm_pool.tile([P, NT], f32, tag="stdv")
                nc.scalar.activation(stdv[:], var[:], AF.Sqrt, bias=eps_sb[:, 0:1])
                rstd = sm_pool.tile([P, NT], f32, tag="rstd")
                nc.vector.reciprocal(rstd[:], stdv[:])

                # z = relu((h - mu) * rstd [* g + b])  (bf16, node-major)
                zf = big_pool.tile([P, NT, H], f32, tag="big")
                nc.gpsimd.tensor_tensor(
                    zf[:, :, :], h[:, :, :], mu[:].broadcast_to([P, NT, H]), op=SUB)
                nc.gpsimd.tensor_tensor(
                    zf[:, :, :], zf[:, :, :], rstd[:].broadcast_to([P, NT, H]), op=MUL)
                if not trivial_affine:
                    nc.vector.tensor_tensor(
                        zf[:, :, :], zf[:, :, :], _bmid(aux_sb[:, l, :], NT), op=MUL)
                    nc.vector.tensor_tensor(
                        zf[:, :, :], zf[:, :, :], _bmid(aux_sb[:, L + l, :], NT), op=ADD)
                nc.scalar.activation(z[:, :, :], zf[:, :, :], AF.Relu)

                # s^T = z^T @ B^T, col-tiled 2x
                st = st_pool.tile([P, N // 2], bf16, tag="st")
                pss = ps_s.tile([P, N // 2], f32)
                for cp in range(2):   # chunk pair: A chunk cp, B chunk 2+cp
                    csA = slice(cp * CHW, (cp + 1) * CHW)
                    csB = slice((2 + cp) * CHW, (3 + cp) * CHW)
                    csP = slice(cp * CHW, (cp + 1) * CHW)
                    for jb in range(NT):
                        nc.tensor.matmul(
                            pss[0:H, csP], lhsT=z[:, jb, :],
                            rhs=bt_sb[:, jb, csA],
                            start=(jb == 0), stop=(jb == NT - 1))
                        nc.tensor.matmul(
                            pss[H:P, csP], lhsT=z[:, jb, :],
                            rhs=bt_sb[:, jb, csB],
                            start=(jb == 0), stop=(jb == NT - 1),
                            skip_group_check=SKIP_GROUP_CHECK_FOR_SIM)
                    if cp == 0:
                        nc.vector.tensor_copy(st[:, csP], pss[:, csP])
                    else:
                        nc.scalar.copy(st[:, csP], pss[:, csP])

                # h += s @ Wt[l]
                for half in range(2):
                    wp = ps_w.tile([P, HALF, H], f32)
                    for q in range(HALF):
                        i = half * HALF + q
                        if i < NT // 2:
                            lhsT = st[0:H, i * P:(i + 1) * P]
                            rhs = pk_sb[0:H, 1 + l, :]
                        else:
                            j = i - NT // 2
                            lhsT = st[H:P, j * P:(j + 1) * P]
                            rhs = pk_sb[H:P, 1 + l, :]
                        nc.tensor.matmul(
                            wp[:, q, :], lhsT=lhsT, rhs=rhs,
                            start=True, stop=True)
                    hs = slice(half * HALF, (half + 1) * HALF)
                    nc.vector.tensor_tensor(
                        h[:, hs, :], h[:, hs, :], wp[:, :, :], op=ADD)

            # ---- head: out = h @ head_w (+ head_b) ----
            th = big_pool.tile([P, NT, H], f32, tag="big")
            nc.gpsimd.tensor_tensor(
                th[:, :, :], h[:, :, :], _bmid(hwv_sb[:, :], NT), op=MUL)
            osb = ob_pool.tile([P, NT], f32)
            nc.vector.tensor_reduce(osb[:], th[:, :, :], axis=AX, op=ADD)
            if not trivial_affine:
                nc.vector.tensor_scalar(
                    osb[:], osb[:], aux_sb[:, 2 * L + 1, 0:1], None, ADD)
            nc.sync.dma_start(
                out=out[g].rearrange("(ib p) one -> p (ib one)", p=P),
                in_=osb[:, :])

    nc.compile()
    return nc


def _build_nc(trivial_affine=True, reps=1, graphs=G):
    if trivial_affine:
        return _build_nc_fast(reps=reps, graphs=graphs)
    return _build_nc_general(trivial_affine=False, reps=reps, graphs=graphs)


_NC = {}


def _get_nc(trivial_affine=True):
    key = trivial_affine
    if key not in _NC:
        _NC[key] = _build_nc(trivial_affine=trivial_affine)
    return _NC[key]


# Aggregation-chunk node permutation: chunk cp of the dual-chain psum
# must cover nodes [cp*1024, (cp+1)*1024) so each chunk completes one
# full h PSUM bank.  Chain A (psum rows 0:64) streams bt cols
# [cp*512,(cp+1)*512) -> nodes cp*1024..cp*1024+511; chain B (rows
# 64:128) streams cols [1024+cp*512, ...) -> nodes cp*1024+512.. .
_COL_PERM = np.concatenate([
    np.arange(0, 512), np.arange(1024, 1536),
    np.arange(512, 1024), np.arange(1536, 2048)])
# bt col c must hold node perm[c]:
_COL_NODES = np.empty(N, np.int64)
_COL_NODES[0:512] = np.arange(0, 512)           # chain A cp0
_COL_NODES[512:1024] = np.arange(1024, 1536)    # chain A cp1
_COL_NODES[1024:1536] = np.arange(512, 1024)    # chain B cp0
_COL_NODES[1536:2048] = np.arange(1536, 2048)   # chain B cp1


def _prep_in_maps(inputs):
    x = np.asarray(inputs["x"], np.float32)
    adj = np.asarray(inputs["adj"], np.float32)
    proj_w = np.asarray(inputs["proj_w"], np.float32)
    proj_b = np.asarray(inputs["proj_b"], np.float32)
    ln_g = np.asarray(inputs["ln_g"], np.float32)
    ln_b = np.asarray(inputs["ln_b"], np.float32)
    conv_w = np.asarray(inputs["conv_w"], np.float32)
    head_w = np.asarray(inputs["head_w"], np.float32)
    head_b = np.asarray(inputs["head_b"], np.float32)

    trivial_affine = bool(
        np.all(ln_g == 1.0) and np.all(ln_b == 0.0)
        and np.all(proj_b == 0.0) and np.all(head_b == 0.0))

    # Wt[l] = (1-beta) I + beta conv_w[l]
    if trivial_affine:
        # fast path: slot 1+2l = [Wt; 0], slot 2+2l = [0; Wt] so every
        # s@Wt matmul is full-K / tile_position (0,0)
        pkh = np.zeros((P, 1 + 2 * L, H), np.float32)
        pkh[:, 0, :] = proj_w
        for l in range(L):
            beta = math.log(LAMDA / (l + 1) + 1.0)
            wt = (1.0 - beta) * np.eye(H, dtype=np.float32) + beta * conv_w[l]
            pkh[:H, 1 + 2 * l, :] = wt
            pkh[H:, 2 + 2 * l, :] = wt
    else:
        pkh = np.zeros((P, 1 + L, H), np.float32)
        pkh[:, 0, :] = proj_w
        for l in range(L):
            beta = math.log(LAMDA / (l + 1) + 1.0)
            wt = (1.0 - beta) * np.eye(H, dtype=np.float32) + beta * conv_w[l]
            pkh[:H, 1 + l, :] = wt
            pkh[H:, 1 + l, :] = wt

    # B^T per graph, bf16: bt[j, i] = 0.9*d_j*d_i*((A!=0)^T + I)[j,i] + 0.1*I
    ey = np.eye(N, dtype=np.float32)
    bt_all = np.empty((BS, N, N), ml_dtypes.bfloat16)

    def _build_bt(gi):
        pat = adj[gi] != 0
        deg = pat.sum(-1, dtype=np.float32) + 1.0
        sc = np.sqrt(0.9 / deg)                         # sqrt(0.9) * deg^-1/2
        b = pat.T.astype(np.float32)
        b += ey
        b *= sc[:, None]
        b *= sc[None, :]
        b[np.arange(N), np.arange(N)] += 0.1
        if trivial_affine:
            b = b[:, _COL_NODES]                        # fast-path col perm
        bt_all[gi] = b.astype(ml_dtypes.bfloat16)

    from concurrent.futures import ThreadPoolExecutor
    with ThreadPoolExecutor(max_workers=8) as ex:
        list(ex.map(_build_bt, range(BS)))

    xt_all = np.ascontiguousarray(
        x.transpose(0, 2, 1)).astype(ml_dtypes.bfloat16)

    shared = {
        "pk": pkh.astype(ml_dtypes.bfloat16),
        "hwv": np.ascontiguousarray(
            np.broadcast_to(head_w[:, 0][None, :], (P, H))),
    }
    if not trivial_affine:
        aux = np.zeros((P, 2 * L + 2, H), np.float32)
        aux[:, :L, :] = np.broadcast_to(ln_g[:, None, :], (L, P, H)).transpose(1, 0, 2)
        aux[:, L:2 * L, :] = np.broadcast_to(ln_b[:, None, :], (L, P, H)).transpose(1, 0, 2)
        aux[:, 2 * L, :] = proj_b[None, :]
        aux[:, 2 * L + 1, 0] = head_b[0]
        shared["aux"] = aux

    in_maps = []
    for c in range(NCORES):
        sl = slice(c * G, (c + 1) * G)
        in_maps.append(dict(
            shared,
            bt=np.ascontiguousarray(bt_all[sl]),
            xt=np.ascontiguousarray(xt_all[sl]),
        ))
    return in_maps, trivial_affine


def kernel(**inputs) -> np.ndarray:
    in_maps, trivial_affine = _prep_in_maps(inputs)
    nc = _get_nc(trivial_affine)
    res = run_bass_kernel_spmd(nc, in_maps, list(range(NCORES)))
    return np.concatenate([res.results[c]["out"] for c in range(NCORES)], axis=0)
